# revision 1
# baseline (speedup 1.0000x reference)
"""Two-layer GATv2 GNN (N=50000, E=800000, 128->4x32->64) on 8 Trainium2
NeuronCores.

Strategy
--------
Host: add self-loops, sort edges by dst, shard dst nodes contiguously across 8
cores (6250 each). Per core, nodes are grouped into 49 "supertiles" of 128
consecutive dst nodes; each supertile's incoming edges are packed into B blocks
of 128 edges (padded; padding edges get an out-of-range slot so they aggregate
to nothing).

Device, per layer:
  dense:  xl = x @ Wl (+ fused per-head att-dot columns), xr likewise
  edges:  gather xl[src] rows, DMA-accumulate xr[dst] rows -> z
          logits = 0.8*att.relu(z) (reduce) + 0.2*(att.z) (prefused lin cols)
          w = exp(logits)  (softmax denominators aggregated alongside, no
          two-pass segment softmax needed)
          one-hot slot matrix S built with is_equal against an iota row
          PE matmul S^T @ [w*xl | w] accumulates per-node sums in PSUM
  epilogue: divide by denominator (+1e-16), bias, ELU (layer1), write out.
Between layers one AllGather shares the dense-transformed xl2 across cores.
All output writes are static DMAs (every node has a self-loop, so supertiles
cover contiguous node ranges).
"""
import numpy as np

import concourse.bass as bass
import concourse.mybir as mybir
from concourse.tile import TileContext
from concourse.masks import make_identity
from concourse.bass_utils import run_bass_kernel_spmd

# ---------------- problem constants ----------------
N = 50000
IN = 128
HID = 32
HEADS = 4
H1 = HEADS * HID       # 128
OUT = 64
NCORES = 8
P = 128
PAD_SLOT = 200.0
GROUPED_GATHERS = False

F32 = mybir.dt.float32
I32 = mybir.dt.int32
AF = mybir.ActivationFunctionType
ALU = mybir.AluOpType


# ------------- walrus workaround -------------
def split_multi_waits(nc):
    """This environment's walrus build rejects any instruction carrying more
    than one sem wait ("Too many sync wait commands"). Move extra waits onto
    engine NOPs inserted immediately before the instruction."""
    import bass_rust
    for f in nc.m.functions:
        for blk in f.blocks:
            il = blk.instructions
            i = 0
            while i < len(il):
                inst = il[i]
                si = inst.sync_info
                if si is not None and si.on_wait is not None and len(si.on_wait) > 1:
                    waits = list(si.on_wait)
                    si.on_wait = waits[-1:]
                    for w in waits[:-1]:
                        nop = nc.engines[inst.engine].nop(nofuse=True).ins
                        cur = nc.cur_bb.bb.instructions
                        assert cur[-1] is nop
                        cur.pop()
                        nop.sync_info = bass_rust.SyncInfo(on_wait=[w], on_update=[])
                        il.insert(i, nop)
                        i += 1
                i += 1


# ---------------- host preprocessing ----------------
def prep(inputs, n=N, ncores=NCORES):
    """Returns (in_maps, B). Shapes are data-driven only through B."""
    nloc = n // ncores
    st_n = (nloc + P - 1) // P
    x = np.ascontiguousarray(np.asarray(inputs["x"], dtype=np.float32))
    ei = np.asarray(inputs["edge_index"])
    W1_l = np.asarray(inputs["W1_l"], np.float32)
    W1_r = np.asarray(inputs["W1_r"], np.float32)
    b1 = np.asarray(inputs["b1"], np.float32)
    att1 = np.asarray(inputs["att1"], np.float32)
    W2_l = np.asarray(inputs["W2_l"], np.float32)
    W2_r = np.asarray(inputs["W2_r"], np.float32)
    b2 = np.asarray(inputs["b2"], np.float32)
    att2 = np.asarray(inputs["att2"], np.float32)

    loop = np.arange(n, dtype=np.int64)
    s_all = np.concatenate([ei[0].astype(np.int64), loop])
    d_all = np.concatenate([ei[1].astype(np.int64), loop])
    order = np.argsort(d_all, kind="stable")
    s_all = s_all[order].astype(np.int32)
    d_all = d_all[order].astype(np.int32)

    bounds = np.searchsorted(d_all, np.arange(ncores + 1) * nloc)
    # first pass: per-supertile block counts (max over cores)
    blocks = np.ones(st_n, np.int64)
    core_data = []
    for c in range(ncores):
        lo, hi = bounds[c], bounds[c + 1]
        dl = d_all[lo:hi] - c * nloc
        sl = s_all[lo:hi]
        stc = dl >> 7
        counts = np.bincount(stc, minlength=st_n)
        blocks = np.maximum(blocks, (counts + P - 1) // P)
        core_data.append((dl, sl, stc, counts))
    B = int(blocks.max())

    # weights / consts
    A1 = np.zeros((H1, HEADS), np.float32)
    for h in range(HEADS):
        A1[h * HID:(h + 1) * HID, h] = att1[h]
    Wa1_l = 0.2 * (W1_l @ A1)
    Wa1_r = 0.2 * (W1_r @ A1)
    W1cat = np.concatenate([W1_l, Wa1_l, W1_r, Wa1_r], axis=1).astype(np.float32)
    A2 = att2.reshape(OUT, 1).astype(np.float32)
    Wa2_l = 0.2 * (W2_l @ A2)
    Wa2_r = 0.2 * (W2_r @ A2)
    zc = np.zeros((H1, 1), np.float32)
    W2cat = np.concatenate([W2_l, Wa2_l, zc, W2_r, Wa2_r, zc], axis=1).astype(np.float32)
    att1r = np.tile(0.8 * att1.reshape(1, H1), (P, 1)).astype(np.float32)
    att2r = np.tile(0.8 * att2.reshape(1, OUT), (P, 1)).astype(np.float32)
    b1r = np.tile(b1.reshape(1, H1), (P, 1)).astype(np.float32)
    b2r = np.tile(b2.reshape(1, OUT), (P, 1)).astype(np.float32)
    colix = np.tile(np.arange(P, dtype=np.float32), (P, 1))
    xT = np.ascontiguousarray(x.T)

    in_maps = []
    for c in range(ncores):
        dl, sl, stc, counts = core_data[c]
        starts = np.zeros(st_n, np.int64)
        starts[1:] = np.cumsum(counts)[:-1]
        pos = np.arange(len(dl)) - starts[stc]
        bb = (pos >> 7).astype(np.int64)
        ee = (pos & 127).astype(np.int64)
        esrc = np.zeros((st_n, P, B), np.int32)
        edst = np.zeros((st_n, P, B), np.int32)
        ek = np.full((st_n, P, B), PAD_SLOT, np.float32)
        esrc[stc, ee, bb] = sl
        edst[stc, ee, bb] = dl
        ek[stc, ee, bb] = (dl - (stc << 7)).astype(np.float32)
        edat = np.concatenate([esrc, edst, ek.view(np.int32)], axis=2)
        in_maps.append({
            "xT": xT,
            "xTo": np.ascontiguousarray(x[c * nloc:(c + 1) * nloc].T),
            "W1": W1cat, "W2": W2cat,
            "att1r": att1r, "att2r": att2r,
            "b1r": b1r, "b2r": b2r, "colix": colix,
            "edat": edat,
        })
    return in_maps, blocks


# ---------------- device program ----------------
def build_program(blocks, n=N, ncores=NCORES, grp=8, reps=1):
    blocks = [int(b) for b in blocks]
    B = max(blocks)
    nloc = n // ncores
    st_n = (nloc + P - 1) // P
    last = nloc - (st_n - 1) * P
    w1row = H1 + HEADS          # 132: [xl | a_l]
    w2row = OUT + 2             # 66:  [xl2 | a2l | pad]
    nt_full = (n + P - 1) // P  # dense tiles over all nodes

    nc = bass.Bass()
    xT = nc.dram_tensor("xT", [P, n], F32, kind="ExternalInput")
    xTo = nc.dram_tensor("xTo", [P, nloc], F32, kind="ExternalInput")
    W1 = nc.dram_tensor("W1", [P, 2 * w1row], F32, kind="ExternalInput")
    W2 = nc.dram_tensor("W2", [P, 2 * w2row], F32, kind="ExternalInput")
    att1r = nc.dram_tensor("att1r", [P, H1], F32, kind="ExternalInput")
    att2r = nc.dram_tensor("att2r", [P, OUT], F32, kind="ExternalInput")
    b1r = nc.dram_tensor("b1r", [P, H1], F32, kind="ExternalInput")
    b2r = nc.dram_tensor("b2r", [P, OUT], F32, kind="ExternalInput")
    colix = nc.dram_tensor("colix", [P, P], F32, kind="ExternalInput")
    edat = nc.dram_tensor("edat", [st_n, P, 3 * B], I32, kind="ExternalInput")
    out_loc = nc.dram_tensor("out_loc", [nloc, OUT], F32, kind="ExternalOutput")

    xl1 = nc.dram_tensor("xl1", [n, w1row], F32)
    xr1 = nc.dram_tensor("xr1", [nloc, w1row], F32)
    hT = nc.dram_tensor("hT", [P, nloc], F32)
    xl2g = nc.dram_tensor("xl2g", [nloc, w2row], F32)
    xl2 = nc.dram_tensor("xl2", [n, w2row], F32, addr_space="Shared")
    xr2 = nc.dram_tensor("xr2", [nloc, w2row], F32)

    cc_sem = nc.alloc_semaphore("cc_sem")

    rep_emit = []

    def edge_layer(tc, pools, consts, table_l, table_r, row_w, dat_w, heads,
                   att_sb, bias_sb, layer):
        """Shared edge-phase emitter for both layers."""
        pool, psum_agg, psum_tp = pools
        colix_sb, ident = consts["colix"], consts["ident"]
        mrow = dat_w + heads  # matmul rhs width per block
        for st in range(st_n):
            cnt = P if st < st_n - 1 else last
            bst = blocks[st]
            edt = pool.tile([P, 3 * bst], I32, tag="edt")
            nc.scalar.dma_start(
                out=edt[:].rearrange("p (k b) -> p k b", k=3),
                in_=edat[st].rearrange("p (k b) -> p k b", k=3)[:, :, 0:bst])
            es = edt[:, 0:bst]
            ed = edt[:, bst:2 * bst]
            ekt = edt[:, 2 * bst:3 * bst].bitcast(F32)
            ps = psum_agg.tile([P, mrow], F32, tag="agg")
            for g0 in range(0, bst, grp):
                gw = min(grp, bst - g0)
                xz = pool.tile([P, gw * row_w], F32, tag="xz")
                xz3g = xz[:].rearrange("p (g w) -> p g w", w=row_w)
                if GROUPED_GATHERS:
                    nc.gpsimd.indirect_dma_start(
                        out=xz3g, out_offset=None, in_=table_l[:],
                        in_offset=bass.IndirectOffsetOnAxis(
                            ap=es[:, g0:g0 + gw], axis=0))
                    nc.gpsimd.indirect_dma_start(
                        out=xz3g, out_offset=None, in_=table_r[:],
                        in_offset=bass.IndirectOffsetOnAxis(
                            ap=ed[:, g0:g0 + gw], axis=0),
                        compute_op=ALU.add)
                else:
                    for b in range(gw):
                        sl = xz[:, b * row_w:(b + 1) * row_w]
                        nc.gpsimd.indirect_dma_start(
                            out=sl, out_offset=None, in_=table_l[:],
                            in_offset=bass.IndirectOffsetOnAxis(
                                ap=es[:, g0 + b:g0 + b + 1], axis=0))
                        nc.gpsimd.indirect_dma_start(
                            out=sl, out_offset=None, in_=table_r[:],
                            in_offset=bass.IndirectOffsetOnAxis(
                                ap=ed[:, g0 + b:g0 + b + 1], axis=0),
                            compute_op=ALU.add)
                r4 = pool.tile([P, gw * row_w], F32, tag="r4")
                nc.scalar.activation(r4[:], xz[:], AF.Relu)
                xz3 = xz[:].rearrange("p (g w) -> p g w", w=row_w)
                r43 = r4[:].rearrange("p (g w) -> p g w", w=row_w)
                pr = pool.tile([P, gw * dat_w], F32, tag="pr")
                nc.vector.tensor_tensor(
                    out=pr[:].rearrange("p (g w) -> p g w", w=dat_w),
                    in0=r43[:, :, 0:dat_w],
                    in1=att_sb[:, None, :].to_broadcast([P, gw, dat_w]),
                    op=ALU.mult)
                lg = pool.tile([P, gw * heads], F32, tag="lg")
                nc.vector.reduce_sum(
                    out=lg[:].rearrange("p (g h) -> p g h", h=heads),
                    in_=pr[:].rearrange("p (g h c) -> p g h c",
                                        h=heads, c=dat_w // heads),
                    axis=mybir.AxisListType.X)
                lgf = pool.tile([P, gw * heads], F32, tag="lgf")
                nc.vector.tensor_tensor(
                    out=lgf[:].rearrange("p (g h) -> p g h", h=heads),
                    in0=lg[:].rearrange("p (g h) -> p g h", h=heads),
                    in1=xz3[:, :, dat_w:dat_w + heads],
                    op=ALU.add)
                mg = pool.tile([P, gw * mrow], F32, tag="mg")
                mg3 = mg[:].rearrange("p (g w) -> p g w", w=mrow)
                nc.scalar.activation(
                    mg3[:, :, dat_w:dat_w + heads],
                    lgf[:].rearrange("p (g h) -> p g h", h=heads),
                    AF.Exp)
                cph = dat_w // heads
                nc.vector.tensor_tensor(
                    out=mg[:].rearrange("p (g m) -> p g m", m=mrow)
                        [:, :, 0:dat_w].rearrange("p g (h c) -> p g h c", c=cph),
                    in0=xz3[:, :, 0:dat_w].rearrange("p g (h c) -> p g h c", c=cph),
                    in1=mg3[:, :, dat_w:dat_w + heads][:, :, :, None]
                        .to_broadcast([P, gw, heads, cph]),
                    op=ALU.mult)
                s4 = pool.tile([P, gw * P], F32, tag="s4")
                nc.vector.tensor_tensor(
                    out=s4[:].rearrange("p (g q) -> p g q", q=P),
                    in0=colix_sb[:, None, :].to_broadcast([P, gw, P]),
                    in1=ekt[:, g0:g0 + gw, None].to_broadcast([P, gw, P]),
                    op=ALU.is_equal)
                for b in range(gw):
                    nc.tensor.matmul(
                        out=ps[:],
                        lhsT=s4[:, b * P:(b + 1) * P],
                        rhs=mg[:, b * mrow:(b + 1) * mrow],
                        start=(g0 + b == 0), stop=(g0 + b == bst - 1))
            # ---- epilogue ----
            # The fused gather accumulated z = xl[src] + xr[dst]; per node i the
            # aggregate is sum(w*xl_src) + xr_i*sum(w), so subtract xr_i*sum(w).
            xrn = pool.tile([P, dat_w], F32, tag="xrn")
            if cnt < P:
                nc.gpsimd.memset(xrn[:], 0.0)
            nc.scalar.dma_start(out=xrn[:cnt, :],
                              in_=table_r[st * P:st * P + cnt, 0:dat_w])
            dn = pool.tile([P, heads], F32, tag="dn")
            nc.vector.tensor_scalar_add(dn[:], ps[:, dat_w:dat_w + heads], 1e-16)
            r0 = pool.tile([P, heads], F32, tag="r0")
            nc.vector.reciprocal(r0[:], dn[:])
            e1 = pool.tile([P, heads], F32, tag="e1")
            nc.vector.tensor_tensor(out=e1[:], in0=r0[:], in1=dn[:], op=ALU.mult)
            t2 = pool.tile([P, heads], F32, tag="t2")
            nc.vector.tensor_scalar(out=t2[:], in0=e1[:], scalar1=-1.0,
                                    scalar2=2.0, op0=ALU.mult, op1=ALU.add)
            r1 = pool.tile([P, heads], F32, tag="r1")
            nc.vector.tensor_tensor(out=r1[:], in0=r0[:], in1=t2[:], op=ALU.mult)
            cor = pool.tile([P, dat_w], F32, tag="cor")
            nc.vector.tensor_tensor(
                out=cor[:].rearrange("p (h c) -> p h c", c=cph),
                in0=xrn[:].rearrange("p (h c) -> p h c", c=cph),
                in1=dn[:, :, None].to_broadcast([P, heads, cph]),
                op=ALU.mult)
            sub = pool.tile([P, dat_w], F32, tag="sub")
            nc.vector.tensor_tensor(out=sub[:], in0=ps[:, 0:dat_w], in1=cor[:],
                                    op=ALU.subtract)
            ob = pool.tile([P, dat_w], F32, tag="ob")
            nc.vector.tensor_tensor(
                out=ob[:].rearrange("p (h c) -> p h c", c=cph),
                in0=sub[:].rearrange("p (h c) -> p h c", c=cph),
                in1=r1[:, :, None].to_broadcast([P, heads, cph]),
                op=ALU.mult)
            ob2 = pool.tile([P, dat_w], F32, tag="ob2")
            nc.vector.tensor_tensor(out=ob2[:], in0=ob[:], in1=bias_sb[:], op=ALU.add)
            if layer == 1:
                mn = pool.tile([P, dat_w], F32, tag="mn")
                nc.vector.tensor_scalar_min(mn[:], ob2[:], 0.0)
                ex = pool.tile([P, dat_w], F32, tag="ex")
                nc.scalar.activation(ex[:], mn[:], AF.Exp)
                rl = pool.tile([P, dat_w], F32, tag="rl")
                nc.scalar.activation(rl[:], ob2[:], AF.Relu)
                sm = pool.tile([P, dat_w], F32, tag="sm")
                nc.vector.tensor_tensor(out=sm[:], in0=ex[:], in1=rl[:], op=ALU.add)
                he = pool.tile([P, dat_w], F32, tag="he")
                nc.vector.tensor_scalar_add(he[:], sm[:], -1.0)
                tp = psum_tp.tile([P, P], F32, tag="tp")
                nc.tensor.transpose(out=tp[:], in_=he[:], identity=ident[:])
                ts = pool.tile([P, P], F32, tag="ts")
                nc.scalar.copy(out=ts[:], in_=tp[:])
                nc.sync.dma_start(out=hT[:, st * P:st * P + cnt], in_=ts[:, :cnt])
            else:
                nc.sync.dma_start(out=out_loc[st * P:st * P + cnt, :],
                                  in_=ob2[:cnt, :])

    for rep in range(reps):
        # one TileContext: dense1 + edges1 + dense2 + AllGather + edges2.
        # Tile's shadow-memory tracks DRAM deps, so the collective and both
        # edge phases order correctly while unrelated work overlaps.
        with TileContext(nc) as tc:
            with tc.tile_pool(name="const", bufs=1) as cpool, \
                 tc.tile_pool(name="work", bufs=4) as pool, \
                 tc.tile_pool(name="dense", bufs=4) as dpool, \
                 tc.tile_pool(name="pagg", bufs=2, space="PSUM") as psum_agg, \
                 tc.tile_pool(name="ptp", bufs=2, space="PSUM") as psum_tp, \
                 tc.tile_pool(name="pd", bufs=4, space="PSUM") as psum_d:
                w1_sb = cpool.tile([P, 2 * w1row], F32)
                nc.sync.dma_start(out=w1_sb[:], in_=W1[:])
                w2_sb = cpool.tile([P, 2 * w2row], F32)
                nc.sync.dma_start(out=w2_sb[:], in_=W2[:])
                att1_sb = cpool.tile([P, H1], F32)
                nc.sync.dma_start(out=att1_sb[:], in_=att1r[:])
                att2_sb = cpool.tile([P, OUT], F32)
                nc.sync.dma_start(out=att2_sb[:], in_=att2r[:])
                b1_sb = cpool.tile([P, H1], F32)
                nc.sync.dma_start(out=b1_sb[:], in_=b1r[:])
                b2_sb = cpool.tile([P, OUT], F32)
                nc.sync.dma_start(out=b2_sb[:], in_=b2r[:])
                colix_sb = cpool.tile([P, P], F32)
                nc.sync.dma_start(out=colix_sb[:], in_=colix[:])
                ident = cpool.tile([P, P], F32)
                make_identity(nc, ident[:])
                consts = {"colix": colix_sb, "ident": ident}

                # dense-1: xl1 (all nodes), batched 4 tiles per DMA
                nb = 4
                for t0 in range(0, nt_full, nb):
                    k_n = min(nb, nt_full - t0)
                    cols_all = min(P * k_n, n - t0 * P)
                    xt = dpool.tile([P, P * k_n], F32, tag="xt4")
                    nc.scalar.dma_start(out=xt[:, :cols_all],
                                        in_=xT[:, t0 * P:t0 * P + cols_all])
                    sb = dpool.tile([P, k_n * w1row], F32, tag="sbd4")
                    for k in range(k_n):
                        cols = min(P, n - (t0 + k) * P)
                        psd = psum_d.tile([cols, w1row], F32, tag="psd")
                        nc.tensor.matmul(out=psd[:],
                                         lhsT=xt[:, k * P:k * P + cols],
                                         rhs=w1_sb[:, 0:w1row],
                                         start=True, stop=True)
                        nc.scalar.copy(out=sb[:cols, k * w1row:(k + 1) * w1row],
                                       in_=psd[:])
                    rows = min(P * k_n, n - t0 * P)
                    if rows == P * k_n:
                        nc.sync.dma_start(
                            out=xl1[t0 * P:t0 * P + rows, :]
                                .rearrange("(k p) w -> p k w", p=P),
                            in_=sb[:].rearrange("p (k w) -> p k w", w=w1row))
                    else:
                        # ragged tail: per-block writes
                        for k in range(k_n):
                            cols = min(P, n - (t0 + k) * P)
                            nc.sync.dma_start(
                                out=xl1[(t0 + k) * P:(t0 + k) * P + cols, :],
                                in_=sb[:cols, k * w1row:(k + 1) * w1row])
                for t in range(st_n):
                    cols = P if t < st_n - 1 else last
                    xt = dpool.tile([P, cols], F32, tag="xt")
                    nc.scalar.dma_start(out=xt[:], in_=xTo[:, t * P:t * P + cols])
                    psd = psum_d.tile([cols, w1row], F32, tag="psd")
                    nc.tensor.matmul(out=psd[:], lhsT=xt[:],
                                     rhs=w1_sb[:, w1row:2 * w1row],
                                     start=True, stop=True)
                    sb = dpool.tile([cols, w1row], F32, tag="sbd")
                    nc.scalar.copy(out=sb[:], in_=psd[:])
                    nc.sync.dma_start(out=xr1[t * P:t * P + cols, :], in_=sb[:])

                # edges layer 1
                edge_layer(tc, (pool, psum_agg, psum_tp), consts, xl1, xr1,
                           w1row, H1, HEADS, att1_sb, b1_sb, layer=1)

                # dense-2: xl2g + xr2 from hT
                for t in range(st_n):
                    cols = P if t < st_n - 1 else last
                    xh = dpool.tile([P, cols], F32, tag="xt")
                    nc.scalar.dma_start(out=xh[:], in_=hT[:, t * P:t * P + cols])
                    psd2 = psum_d.tile([cols, 2 * w2row], F32, tag="psd")
                    nc.tensor.matmul(out=psd2[:], lhsT=xh[:], rhs=w2_sb[:],
                                     start=True, stop=True)
                    sb2 = dpool.tile([cols, 2 * w2row], F32, tag="sbd")
                    nc.scalar.copy(out=sb2[:], in_=psd2[:])
                    nc.sync.dma_start(out=xl2g[t * P:t * P + cols, :],
                                      in_=sb2[:, 0:w2row])
                    nc.sync.dma_start(out=xr2[t * P:t * P + cols, :],
                                      in_=sb2[:, w2row:2 * w2row])

        # ---- AllGather xl2g -> xl2 (between TileContexts; raw sem) ----
        nc.gpsimd.collective_compute(
            "AllGather", ALU.bypass,
            replica_groups=[list(range(ncores))],
            ins=[xl2g[:]], outs=[xl2[:]],
        ).then_inc(cc_sem)
        nc.gpsimd.wait_ge(cc_sem, rep + 1)

        # ---- TC2: edges layer 2 ----
        with TileContext(nc) as tc:
            with tc.tile_pool(name="const2", bufs=1) as cpool, \
                 tc.tile_pool(name="work2", bufs=4) as pool, \
                 tc.tile_pool(name="pagg2", bufs=2, space="PSUM") as psum_agg, \
                 tc.tile_pool(name="ptp2", bufs=2, space="PSUM") as psum_tp:
                att2_sb = cpool.tile([P, OUT], F32)
                nc.sync.dma_start(out=att2_sb[:], in_=att2r[:])
                b2_sb = cpool.tile([P, OUT], F32)
                nc.sync.dma_start(out=b2_sb[:], in_=b2r[:])
                colix_sb = cpool.tile([P, P], F32)
                nc.sync.dma_start(out=colix_sb[:], in_=colix[:])
                ident = cpool.tile([P, P], F32)
                make_identity(nc, ident[:])
                consts = {"colix": colix_sb, "ident": ident}
                edge_layer(tc, (pool, psum_agg, psum_tp), consts, xl2, xr2,
                           w2row, OUT, 1, att2_sb, b2_sb, layer=2)

    return nc


# ---------------- entry point ----------------
def kernel(**inputs) -> np.ndarray:
    in_maps, blocks = prep(inputs)
    nc = build_program(blocks)
    split_multi_waits(nc)
    res = run_bass_kernel_spmd(nc, in_maps, list(range(NCORES)))
    out = np.concatenate([res.results[c]["out_loc"] for c in range(NCORES)], axis=0)
    return out.astype(np.float32)



# revision 19
# speedup vs baseline: 2.4135x; 2.4135x over previous
"""Two-layer GATv2 GNN (N=50000, E=800000, 128->4x32->64) on 8 Trainium2
NeuronCores.

Strategy
--------
Host: add self-loops, sort edges by dst, shard dst nodes contiguously across 8
cores (6250 each). Per core, nodes are grouped into 49 "supertiles" of 128
consecutive dst nodes; each supertile's incoming edges are packed into B blocks
of 128 edges (padded; padding edges get an out-of-range slot so they aggregate
to nothing).

Device, per layer:
  dense:  xl = x @ Wl (+ fused per-head att-dot columns), xr likewise
  edges:  gather xl[src] rows (one indirect DMA per 128-edge block; the
          SWDGE fixed cost ~1us/call on Pool is the kernel's floor).
          The xr[dst] side needs NO per-edge DMA: dst slots are supertile-
          local, so a PE outer product broadcasts the host-packed slot row
          (ekt), is_equal builds the transposed one-hot S^T, and
          z = S^T.T@xr + I@xg is accumulated directly in PSUM by two
          matmuls per block (no DVE add).
          logits = 0.8*att.relu(z) (reduce) + 0.2*(att.z) (prefused lin cols)
          w = exp(logits)  (softmax denominators aggregated alongside, no
          two-pass segment softmax needed)
          one-hot slot matrix S built with is_equal against an iota row
          PE matmul S^T @ [w*z | w] accumulates per-node sums in PSUM
  epilogue: subtract xr_i*sum(w) (z includes xr), divide by denominator,
          bias, ELU (layer1), write out.
Between layers one AllGather shares the dense-transformed xl2 across cores.
All output writes are static DMAs (every node has a self-loop, so supertiles
cover contiguous node ranges).

Known dead ends on this HW/toolchain (do not retry): multi-offset
indirect_dma_start (offsets [P,k>1]) crashes or corrupts; dma_gather /
GPSIMD ucode library ops fail to compile (load_library -> "ISA wrong
length"); so one indirect DMA per 128 edges is the minimum gather cost.
"""
import numpy as np

import concourse.bass as bass
import concourse.mybir as mybir
from concourse.tile import TileContext
from concourse.masks import make_identity
from concourse.bass_utils import run_bass_kernel_spmd

# ---------------- problem constants ----------------
N = 50000
IN = 128
HID = 32
HEADS = 4
H1 = HEADS * HID       # 128
OUT = 64
NCORES = 8
P = 128
PAD_SLOT = 200.0
GROUPED_GATHERS = False

F32 = mybir.dt.float32
I32 = mybir.dt.int32
AF = mybir.ActivationFunctionType
ALU = mybir.AluOpType


# ------------- walrus workaround -------------
def split_multi_waits(nc):
    """This environment's walrus build rejects any instruction carrying more
    than one sem wait ("Too many sync wait commands"). Move extra waits onto
    engine NOPs inserted immediately before the instruction."""
    import bass_rust
    for f in nc.m.functions:
        for blk in f.blocks:
            il = blk.instructions
            i = 0
            while i < len(il):
                inst = il[i]
                si = inst.sync_info
                if si is not None and si.on_wait is not None and len(si.on_wait) > 1:
                    waits = list(si.on_wait)
                    si.on_wait = waits[-1:]
                    for w in waits[:-1]:
                        nop = nc.engines[inst.engine].nop(nofuse=True).ins
                        cur = nc.cur_bb.bb.instructions
                        assert cur[-1] is nop
                        cur.pop()
                        nop.sync_info = bass_rust.SyncInfo(on_wait=[w], on_update=[])
                        il.insert(i, nop)
                        i += 1
                i += 1


# ---------------- host preprocessing ----------------
def prep(inputs, n=N, ncores=NCORES):
    """Returns (in_maps, B). Shapes are data-driven only through B."""
    nloc = n // ncores
    st_n = (nloc + P - 1) // P
    x = np.ascontiguousarray(np.asarray(inputs["x"], dtype=np.float32))
    ei = np.asarray(inputs["edge_index"])
    W1_l = np.asarray(inputs["W1_l"], np.float32)
    W1_r = np.asarray(inputs["W1_r"], np.float32)
    b1 = np.asarray(inputs["b1"], np.float32)
    att1 = np.asarray(inputs["att1"], np.float32)
    W2_l = np.asarray(inputs["W2_l"], np.float32)
    W2_r = np.asarray(inputs["W2_r"], np.float32)
    b2 = np.asarray(inputs["b2"], np.float32)
    att2 = np.asarray(inputs["att2"], np.float32)

    loop = np.arange(n, dtype=np.int64)
    s_all = np.concatenate([ei[0].astype(np.int64), loop])
    d_all = np.concatenate([ei[1].astype(np.int64), loop])
    order = np.argsort(d_all, kind="stable")
    s_all = s_all[order].astype(np.int32)
    d_all = d_all[order].astype(np.int32)

    bounds = np.searchsorted(d_all, np.arange(ncores + 1) * nloc)
    # first pass: per-supertile block counts (max over cores)
    blocks = np.ones(st_n, np.int64)
    core_data = []
    for c in range(ncores):
        lo, hi = bounds[c], bounds[c + 1]
        dl = d_all[lo:hi] - c * nloc
        sl = s_all[lo:hi]
        stc = dl >> 7
        counts = np.bincount(stc, minlength=st_n)
        blocks = np.maximum(blocks, (counts + P - 1) // P)
        core_data.append((dl, sl, stc, counts))
    B = int(blocks.max())

    # weights / consts
    A1 = np.zeros((H1, HEADS), np.float32)
    for h in range(HEADS):
        A1[h * HID:(h + 1) * HID, h] = att1[h]
    Wa1_l = 0.2 * (W1_l @ A1)
    Wa1_r = 0.2 * (W1_r @ A1)
    W1cat = np.concatenate([W1_l, Wa1_l, W1_r, Wa1_r], axis=1).astype(np.float32)
    A2 = att2.reshape(OUT, 1).astype(np.float32)
    Wa2_l = 0.2 * (W2_l @ A2)
    Wa2_r = 0.2 * (W2_r @ A2)
    zc = np.zeros((H1, 1), np.float32)
    W2cat = np.concatenate([W2_l, Wa2_l, zc, W2_r, Wa2_r, zc], axis=1).astype(np.float32)
    att1r = np.tile(0.8 * att1.reshape(1, H1), (P, 1)).astype(np.float32)
    att2r = np.tile(0.8 * att2.reshape(1, OUT), (P, 1)).astype(np.float32)
    b1r = np.tile(b1.reshape(1, H1), (P, 1)).astype(np.float32)
    b2r = np.tile(b2.reshape(1, OUT), (P, 1)).astype(np.float32)
    colix = np.tile(np.arange(P, dtype=np.float32), (P, 1))
    rowix = np.arange(P, dtype=np.float32).reshape(P, 1)
    xT = np.ascontiguousarray(x.T)

    in_maps = []
    for c in range(ncores):
        dl, sl, stc, counts = core_data[c]
        starts = np.zeros(st_n, np.int64)
        starts[1:] = np.cumsum(counts)[:-1]
        pos = np.arange(len(dl)) - starts[stc]
        bb = (pos >> 7).astype(np.int64)
        ee = (pos & 127).astype(np.int64)
        esrc = np.zeros((st_n, P, B), np.int32)
        ek = np.full((st_n, P, B), PAD_SLOT, np.float32)
        esrc[stc, ee, bb] = sl
        ek[stc, ee, bb] = (dl - (stc << 7)).astype(np.float32)
        edat = np.concatenate([esrc, ek.view(np.int32)], axis=2)
        # ek transposed to a flat row per supertile: ekt[st, b*128+p] = ek[st, p, b]
        ekt = np.ascontiguousarray(
            ek.transpose(0, 2, 1).reshape(st_n, B * P))
        in_maps.append({
            "xT": xT,
            "xTo": np.ascontiguousarray(x[c * nloc:(c + 1) * nloc].T),
            "W1": W1cat, "W2": W2cat,
            "att1r": att1r, "att2r": att2r,
            "b1r": b1r, "b2r": b2r, "colix": colix,
            "edat": edat, "ekt": ekt, "rowix": rowix,
        })
    return in_maps, blocks


# ---------------- device program ----------------
def build_program(blocks, n=N, ncores=NCORES, grp=3, reps=1):
    blocks = [int(b) for b in blocks]
    B = max(blocks)
    nloc = n // ncores
    st_n = (nloc + P - 1) // P
    last = nloc - (st_n - 1) * P
    w1row = H1 + HEADS          # 132: [xl | a_l]
    w2row = OUT + 2             # 66:  [xl2 | a2l | pad]
    nt_full = (n + P - 1) // P  # dense tiles over all nodes

    nc = bass.Bass()
    xT = nc.dram_tensor("xT", [P, n], F32, kind="ExternalInput")
    xTo = nc.dram_tensor("xTo", [P, nloc], F32, kind="ExternalInput")
    W1 = nc.dram_tensor("W1", [P, 2 * w1row], F32, kind="ExternalInput")
    W2 = nc.dram_tensor("W2", [P, 2 * w2row], F32, kind="ExternalInput")
    att1r = nc.dram_tensor("att1r", [P, H1], F32, kind="ExternalInput")
    att2r = nc.dram_tensor("att2r", [P, OUT], F32, kind="ExternalInput")
    b1r = nc.dram_tensor("b1r", [P, H1], F32, kind="ExternalInput")
    b2r = nc.dram_tensor("b2r", [P, OUT], F32, kind="ExternalInput")
    colix = nc.dram_tensor("colix", [P, P], F32, kind="ExternalInput")
    edat = nc.dram_tensor("edat", [st_n, P, 2 * B], I32, kind="ExternalInput")
    ekt = nc.dram_tensor("ekt", [st_n, B * P], F32, kind="ExternalInput")
    rowix = nc.dram_tensor("rowix", [P, 1], F32, kind="ExternalInput")
    out_loc = nc.dram_tensor("out_loc", [nloc, OUT], F32, kind="ExternalOutput")

    xl1 = nc.dram_tensor("xl1", [n, w1row], F32)
    xr1 = nc.dram_tensor("xr1", [nloc, w1row], F32)
    hT = nc.dram_tensor("hT", [P, nloc], F32)
    xl2g = nc.dram_tensor("xl2g", [nloc, w2row], F32)
    xl2 = nc.dram_tensor("xl2", [n, w2row], F32, addr_space="Shared")
    xr2 = nc.dram_tensor("xr2", [nloc, w2row], F32)

    cc_sem = nc.alloc_semaphore("cc_sem")

    rep_emit = []

    def edge_layer(tc, pools, consts, table_l, table_r, row_w, dat_w, heads,
                   att_sb, bias_sb, layer):
        """Shared edge-phase emitter for both layers."""
        pool, psum_agg, psum_tp, psum_ek, psum_z = pools
        colix_sb, ident = consts["colix"], consts["ident"]
        rowix_sb, ones1 = consts["rowix"], consts["ones1"]
        mrow = dat_w + heads  # matmul rhs width per block
        for st in range(st_n):
            cnt = P if st < st_n - 1 else last
            bst = blocks[st]
            edt = pool.tile([P, 2 * bst], I32, tag="edt")
            nc.scalar.dma_start(
                out=edt[:].rearrange("p (k b) -> p k b", k=2),
                in_=edat[st].rearrange("p (k b) -> p k b", k=2)[:, :, 0:bst])
            es = edt[:, 0:bst]
            ekb = edt[:, bst:2 * bst].bitcast(F32)
            # per-supertile xr rows (slot-indexed); zero tail rows so the
            # S@xr matmul reads clean partitions
            xr_st = pool.tile([P, row_w], F32, tag="xrst")
            if cnt < P:
                nc.gpsimd.memset(xr_st[:], 0.0)
            nc.scalar.dma_start(out=xr_st[:cnt, :],
                                in_=table_r[st * P:st * P + cnt, :])
            ektr = pool.tile([1, bst * P], F32, tag="ektr")
            nc.scalar.dma_start(out=ektr[:], in_=ekt[st:st + 1, 0:bst * P])
            ps = psum_agg.tile([P, mrow], F32, tag="agg")
            for g0 in range(0, bst, grp):
                gw = min(grp, bst - g0)
                xg = pool.tile([P, gw * row_w], F32, tag="xg")
                for b in range(gw):
                    nc.gpsimd.indirect_dma_start(
                        out=xg[:, b * row_w:(b + 1) * row_w],
                        out_offset=None, in_=table_l[:],
                        in_offset=bass.IndirectOffsetOnAxis(
                            ap=es[:, g0 + b:g0 + b + 1], axis=0))
                # broadcast slot-id row across partitions, build S^T one-hot
                ekp = psum_ek.tile([P, gw * P], F32, tag="ekp")
                nc.tensor.matmul(out=ekp[:], lhsT=ones1[:],
                                 rhs=ektr[0:1, g0 * P:(g0 + gw) * P],
                                 start=True, stop=True)
                s4t = pool.tile([P, gw * P], F32, tag="s4t")
                nc.vector.tensor_tensor(
                    out=s4t[:].rearrange("p (g q) -> p g q", q=P),
                    in0=rowix_sb[:, :, None].to_broadcast([P, gw, P]),
                    in1=ekp[:].rearrange("p (g q) -> p g q", q=P),
                    op=ALU.is_equal)
                # z = S@xr + I@xg accumulated in PSUM (no DVE add needed)
                zp = psum_z.tile([P, gw * row_w], F32, tag="zp")
                for b in range(gw):
                    sl = zp[:, b * row_w:(b + 1) * row_w]
                    nc.tensor.matmul(out=sl,
                                     lhsT=s4t[:, b * P:(b + 1) * P],
                                     rhs=xr_st[:], start=True, stop=False)
                    nc.tensor.matmul(out=sl, lhsT=ident[:],
                                     rhs=xg[:, b * row_w:(b + 1) * row_w],
                                     start=False, stop=True)
                r4 = pool.tile([P, gw * row_w], F32, tag="r4")
                nc.scalar.activation(r4[:], zp[:], AF.Relu)
                xz3 = zp[:].rearrange("p (g w) -> p g w", w=row_w)
                r43 = r4[:].rearrange("p (g w) -> p g w", w=row_w)
                pr = pool.tile([P, gw * dat_w], F32, tag="pr")
                nc.vector.tensor_tensor(
                    out=pr[:].rearrange("p (g w) -> p g w", w=dat_w),
                    in0=r43[:, :, 0:dat_w],
                    in1=att_sb[:, None, :].to_broadcast([P, gw, dat_w]),
                    op=ALU.mult)
                lg = pool.tile([P, gw * heads], F32, tag="lg")
                nc.vector.reduce_sum(
                    out=lg[:].rearrange("p (g h) -> p g h", h=heads),
                    in_=pr[:].rearrange("p (g h c) -> p g h c",
                                        h=heads, c=dat_w // heads),
                    axis=mybir.AxisListType.X)
                lgf = pool.tile([P, gw * heads], F32, tag="lgf")
                nc.vector.tensor_tensor(
                    out=lgf[:].rearrange("p (g h) -> p g h", h=heads),
                    in0=lg[:].rearrange("p (g h) -> p g h", h=heads),
                    in1=xz3[:, :, dat_w:dat_w + heads],
                    op=ALU.add)
                mg = pool.tile([P, gw * mrow], F32, tag="mg")
                mg3 = mg[:].rearrange("p (g w) -> p g w", w=mrow)
                nc.scalar.activation(
                    mg3[:, :, dat_w:dat_w + heads],
                    lgf[:].rearrange("p (g h) -> p g h", h=heads),
                    AF.Exp)
                cph = dat_w // heads
                nc.vector.tensor_tensor(
                    out=mg[:].rearrange("p (g m) -> p g m", m=mrow)
                        [:, :, 0:dat_w].rearrange("p g (h c) -> p g h c", c=cph),
                    in0=xz3[:, :, 0:dat_w].rearrange("p g (h c) -> p g h c", c=cph),
                    in1=mg3[:, :, dat_w:dat_w + heads][:, :, :, None]
                        .to_broadcast([P, gw, heads, cph]),
                    op=ALU.mult)
                s4 = pool.tile([P, gw * P], F32, tag="s4")
                nc.vector.tensor_tensor(
                    out=s4[:].rearrange("p (g q) -> p g q", q=P),
                    in0=colix_sb[:, None, :].to_broadcast([P, gw, P]),
                    in1=ekb[:, g0:g0 + gw, None].to_broadcast([P, gw, P]),
                    op=ALU.is_equal)
                for b in range(gw):
                    nc.tensor.matmul(
                        out=ps[:],
                        lhsT=s4[:, b * P:(b + 1) * P],
                        rhs=mg[:, b * mrow:(b + 1) * mrow],
                        start=(g0 + b == 0), stop=(g0 + b == bst - 1))
            # ---- epilogue ----
            # z = xl[src] + xr[dst]; per node i the aggregate is
            # sum(w*xl_src) + xr_i*sum(w), so subtract xr_i*sum(w).
            xrn = xr_st[:, 0:dat_w]
            dn = pool.tile([P, heads], F32, tag="dn")
            nc.vector.tensor_scalar_add(dn[:], ps[:, dat_w:dat_w + heads], 1e-16)
            r0 = pool.tile([P, heads], F32, tag="r0")
            nc.vector.reciprocal(r0[:], dn[:])
            e1 = pool.tile([P, heads], F32, tag="e1")
            nc.vector.tensor_tensor(out=e1[:], in0=r0[:], in1=dn[:], op=ALU.mult)
            t2 = pool.tile([P, heads], F32, tag="t2")
            nc.vector.tensor_scalar(out=t2[:], in0=e1[:], scalar1=-1.0,
                                    scalar2=2.0, op0=ALU.mult, op1=ALU.add)
            r1 = pool.tile([P, heads], F32, tag="r1")
            nc.vector.tensor_tensor(out=r1[:], in0=r0[:], in1=t2[:], op=ALU.mult)
            cor = pool.tile([P, dat_w], F32, tag="cor")
            nc.vector.tensor_tensor(
                out=cor[:].rearrange("p (h c) -> p h c", c=cph),
                in0=xrn[:].rearrange("p (h c) -> p h c", c=cph),
                in1=dn[:, :, None].to_broadcast([P, heads, cph]),
                op=ALU.mult)
            sub = pool.tile([P, dat_w], F32, tag="sub")
            nc.vector.tensor_tensor(out=sub[:], in0=ps[:, 0:dat_w], in1=cor[:],
                                    op=ALU.subtract)
            ob = pool.tile([P, dat_w], F32, tag="ob")
            nc.vector.tensor_tensor(
                out=ob[:].rearrange("p (h c) -> p h c", c=cph),
                in0=sub[:].rearrange("p (h c) -> p h c", c=cph),
                in1=r1[:, :, None].to_broadcast([P, heads, cph]),
                op=ALU.mult)
            ob2 = pool.tile([P, dat_w], F32, tag="ob2")
            nc.vector.tensor_tensor(out=ob2[:], in0=ob[:], in1=bias_sb[:], op=ALU.add)
            if layer == 1:
                mn = pool.tile([P, dat_w], F32, tag="mn")
                nc.vector.tensor_scalar_min(mn[:], ob2[:], 0.0)
                ex = pool.tile([P, dat_w], F32, tag="ex")
                nc.scalar.activation(ex[:], mn[:], AF.Exp)
                rl = pool.tile([P, dat_w], F32, tag="rl")
                nc.scalar.activation(rl[:], ob2[:], AF.Relu)
                sm = pool.tile([P, dat_w], F32, tag="sm")
                nc.vector.tensor_tensor(out=sm[:], in0=ex[:], in1=rl[:], op=ALU.add)
                he = pool.tile([P, dat_w], F32, tag="he")
                nc.vector.tensor_scalar_add(he[:], sm[:], -1.0)
                tp = psum_tp.tile([P, P], F32, tag="tp")
                nc.tensor.transpose(out=tp[:], in_=he[:], identity=ident[:])
                ts = pool.tile([P, P], F32, tag="ts")
                nc.scalar.copy(out=ts[:], in_=tp[:])
                nc.sync.dma_start(out=hT[:, st * P:st * P + cnt], in_=ts[:, :cnt])
            else:
                nc.sync.dma_start(out=out_loc[st * P:st * P + cnt, :],
                                  in_=ob2[:cnt, :])

    for rep in range(reps):
        # one TileContext: dense1 + edges1 + dense2 + AllGather + edges2.
        # Tile's shadow-memory tracks DRAM deps, so the collective and both
        # edge phases order correctly while unrelated work overlaps.
        with TileContext(nc) as tc:
            with tc.tile_pool(name="const", bufs=1) as cpool, \
                 tc.tile_pool(name="work", bufs=4) as pool, \
                 tc.tile_pool(name="dense", bufs=4) as dpool, \
                 tc.tile_pool(name="pagg", bufs=2, space="PSUM") as psum_agg, \
                 tc.tile_pool(name="ptp", bufs=1, space="PSUM") as psum_tp, \
                 tc.tile_pool(name="pek", bufs=1, space="PSUM") as psum_ek, \
                 tc.tile_pool(name="pz", bufs=2, space="PSUM") as psum_z, \
                 tc.tile_pool(name="pd", bufs=2, space="PSUM") as psum_d:
                w1_sb = cpool.tile([P, 2 * w1row], F32)
                nc.sync.dma_start(out=w1_sb[:], in_=W1[:])
                w2_sb = cpool.tile([P, 2 * w2row], F32)
                nc.sync.dma_start(out=w2_sb[:], in_=W2[:])
                att1_sb = cpool.tile([P, H1], F32)
                nc.sync.dma_start(out=att1_sb[:], in_=att1r[:])
                att2_sb = cpool.tile([P, OUT], F32)
                nc.sync.dma_start(out=att2_sb[:], in_=att2r[:])
                b1_sb = cpool.tile([P, H1], F32)
                nc.sync.dma_start(out=b1_sb[:], in_=b1r[:])
                b2_sb = cpool.tile([P, OUT], F32)
                nc.sync.dma_start(out=b2_sb[:], in_=b2r[:])
                colix_sb = cpool.tile([P, P], F32)
                nc.sync.dma_start(out=colix_sb[:], in_=colix[:])
                ident = cpool.tile([P, P], F32)
                make_identity(nc, ident[:])
                rowix_sb = cpool.tile([P, 1], F32)
                nc.sync.dma_start(out=rowix_sb[:], in_=rowix[:])
                ones1 = cpool.tile([1, P], F32)
                nc.gpsimd.memset(ones1[:], 1.0)
                consts = {"colix": colix_sb, "ident": ident,
                          "rowix": rowix_sb, "ones1": ones1}

                # dense-1: xl1 (all nodes), batched 4 tiles per DMA
                nb = 4
                for t0 in range(0, nt_full, nb):
                    k_n = min(nb, nt_full - t0)
                    cols_all = min(P * k_n, n - t0 * P)
                    xt = dpool.tile([P, P * k_n], F32, tag="xt4")
                    nc.scalar.dma_start(out=xt[:, :cols_all],
                                        in_=xT[:, t0 * P:t0 * P + cols_all])
                    sb = dpool.tile([P, k_n * w1row], F32, tag="sbd4")
                    for k in range(k_n):
                        cols = min(P, n - (t0 + k) * P)
                        psd = psum_d.tile([cols, w1row], F32, tag="psd")
                        nc.tensor.matmul(out=psd[:],
                                         lhsT=xt[:, k * P:k * P + cols],
                                         rhs=w1_sb[:, 0:w1row],
                                         start=True, stop=True)
                        nc.scalar.copy(out=sb[:cols, k * w1row:(k + 1) * w1row],
                                       in_=psd[:])
                    rows = min(P * k_n, n - t0 * P)
                    if rows == P * k_n:
                        nc.sync.dma_start(
                            out=xl1[t0 * P:t0 * P + rows, :]
                                .rearrange("(k p) w -> p k w", p=P),
                            in_=sb[:].rearrange("p (k w) -> p k w", w=w1row))
                    else:
                        # ragged tail: per-block writes
                        for k in range(k_n):
                            cols = min(P, n - (t0 + k) * P)
                            nc.sync.dma_start(
                                out=xl1[(t0 + k) * P:(t0 + k) * P + cols, :],
                                in_=sb[:cols, k * w1row:(k + 1) * w1row])
                for t in range(st_n):
                    cols = P if t < st_n - 1 else last
                    xt = dpool.tile([P, cols], F32, tag="xt")
                    nc.scalar.dma_start(out=xt[:], in_=xTo[:, t * P:t * P + cols])
                    psd = psum_d.tile([cols, w1row], F32, tag="psd")
                    nc.tensor.matmul(out=psd[:], lhsT=xt[:],
                                     rhs=w1_sb[:, w1row:2 * w1row],
                                     start=True, stop=True)
                    sb = dpool.tile([cols, w1row], F32, tag="sbd")
                    nc.scalar.copy(out=sb[:], in_=psd[:])
                    nc.sync.dma_start(out=xr1[t * P:t * P + cols, :], in_=sb[:])

                # edges layer 1
                edge_layer(tc, (pool, psum_agg, psum_tp, psum_ek, psum_z),
                           consts, xl1, xr1,
                           w1row, H1, HEADS, att1_sb, b1_sb, layer=1)

                # dense-2: xl2g + xr2 from hT
                for t in range(st_n):
                    cols = P if t < st_n - 1 else last
                    xh = dpool.tile([P, cols], F32, tag="xt")
                    nc.scalar.dma_start(out=xh[:], in_=hT[:, t * P:t * P + cols])
                    psd2 = psum_d.tile([cols, 2 * w2row], F32, tag="psd")
                    nc.tensor.matmul(out=psd2[:], lhsT=xh[:], rhs=w2_sb[:],
                                     start=True, stop=True)
                    sb2 = dpool.tile([cols, 2 * w2row], F32, tag="sbd")
                    nc.scalar.copy(out=sb2[:], in_=psd2[:])
                    nc.sync.dma_start(out=xl2g[t * P:t * P + cols, :],
                                      in_=sb2[:, 0:w2row])
                    nc.sync.dma_start(out=xr2[t * P:t * P + cols, :],
                                      in_=sb2[:, w2row:2 * w2row])

        # ---- AllGather xl2g -> xl2 (between TileContexts; raw sem) ----
        nc.gpsimd.collective_compute(
            "AllGather", ALU.bypass,
            replica_groups=[list(range(ncores))],
            ins=[xl2g[:]], outs=[xl2[:]],
        ).then_inc(cc_sem)
        nc.gpsimd.wait_ge(cc_sem, rep + 1)

        # ---- TC2: edges layer 2 ----
        with TileContext(nc) as tc:
            with tc.tile_pool(name="const2", bufs=1) as cpool, \
                 tc.tile_pool(name="work2", bufs=4) as pool, \
                 tc.tile_pool(name="pagg2", bufs=2, space="PSUM") as psum_agg, \
                 tc.tile_pool(name="ptp2", bufs=1, space="PSUM") as psum_tp, \
                 tc.tile_pool(name="pek2", bufs=1, space="PSUM") as psum_ek, \
                 tc.tile_pool(name="pz2", bufs=2, space="PSUM") as psum_z:
                att2_sb = cpool.tile([P, OUT], F32)
                nc.sync.dma_start(out=att2_sb[:], in_=att2r[:])
                b2_sb = cpool.tile([P, OUT], F32)
                nc.sync.dma_start(out=b2_sb[:], in_=b2r[:])
                colix_sb = cpool.tile([P, P], F32)
                nc.sync.dma_start(out=colix_sb[:], in_=colix[:])
                ident = cpool.tile([P, P], F32)
                make_identity(nc, ident[:])
                rowix_sb = cpool.tile([P, 1], F32)
                nc.sync.dma_start(out=rowix_sb[:], in_=rowix[:])
                ones1 = cpool.tile([1, P], F32)
                nc.gpsimd.memset(ones1[:], 1.0)
                consts = {"colix": colix_sb, "ident": ident,
                          "rowix": rowix_sb, "ones1": ones1}
                edge_layer(tc, (pool, psum_agg, psum_tp, psum_ek, psum_z),
                           consts, xl2, xr2,
                           w2row, OUT, 1, att2_sb, b2_sb, layer=2)

    return nc


# ---------------- entry point ----------------
def kernel(**inputs) -> np.ndarray:
    in_maps, blocks = prep(inputs)
    nc = build_program(blocks)
    split_multi_waits(nc)
    res = run_bass_kernel_spmd(nc, in_maps, list(range(NCORES)))
    out = np.concatenate([res.results[c]["out_loc"] for c in range(NCORES)], axis=0)
    return out.astype(np.float32)



# revision 46
# speedup vs baseline: 2.7059x; 1.1212x over previous
"""Two-layer GATv2 GNN (N=50000, E=800000, 128->4x32->64) on 8 Trainium2
NeuronCores.

Strategy
--------
Host: sort raw edges by dst, shard dst nodes contiguously across 8 cores
(6250 each). Per core, nodes are grouped into 49 "supertiles" of 128
consecutive dst nodes; each supertile's incoming edges are packed into B
blocks of 128 edges (padded; padding edges get an out-of-range slot so they
aggregate to nothing). Self-loops are NOT packed as edges: their sources are
supertile-local, so each supertile gets one extra on-device block with
contiguous xl rows (no indirect DMA) and an identity scatter.
Gather tables (xl1/xl2) and dense inputs (x, W1) are bf16 — halves table
DMA/SBUF and the inter-layer AllGather; xr tables stay fp32 so the epilogue
correction cancels exactly. Rel err ~2e-3 (gate 2e-2).

Device, per layer:
  dense:  xl = x @ Wl (+ fused per-head att-dot columns), xr likewise
  edges:  gather xl[src] rows (one indirect DMA per 128-edge block; the
          SWDGE fixed cost ~1us/call on Pool is the kernel's floor).
          The xr[dst] side needs NO per-edge DMA: dst slots are supertile-
          local, so a PE outer product broadcasts the host-packed slot row
          (ekt), is_equal builds the transposed one-hot S^T, and
          z = S^T.T@xr + I@xg is accumulated directly in PSUM by two
          matmuls per block (no DVE add).
          logits = 0.8*att.relu(z) (reduce) + 0.2*(att.z) (prefused lin cols)
          w = exp(logits)  (softmax denominators aggregated alongside, no
          two-pass segment softmax needed)
          one-hot slot matrix S built with is_equal against an iota row
          PE matmul S^T @ [w*z | w] accumulates per-node sums in PSUM
  epilogue: subtract xr_i*sum(w) (z includes xr), divide by denominator,
          bias, ELU (layer1), write out.
Between layers one AllGather shares the dense-transformed xl2 across cores.
All output writes are static DMAs (every node has a self-loop, so supertiles
cover contiguous node ranges).

Known dead ends on this HW/toolchain (do not retry): multi-offset
indirect_dma_start (offsets [P,k>1]) crashes or corrupts; dma_gather /
GPSIMD ucode library ops fail to compile (load_library -> "ISA wrong
length"); so one indirect DMA per 128 edges is the minimum gather cost.
"""
import numpy as np

import concourse.bass as bass
import concourse.mybir as mybir
from concourse.tile import TileContext
from concourse.masks import make_identity
from concourse.bass_utils import run_bass_kernel_spmd

# ---------------- problem constants ----------------
N = 50000
IN = 128
HID = 32
HEADS = 4
H1 = HEADS * HID       # 128
OUT = 64
NCORES = 8
P = 128
PAD_SLOT = 200.0
GROUPED_GATHERS = False

F32 = mybir.dt.float32
BF16 = mybir.dt.bfloat16
I32 = mybir.dt.int32
AF = mybir.ActivationFunctionType
ALU = mybir.AluOpType


# ------------- walrus workaround -------------
def split_multi_waits(nc):
    """This environment's walrus build rejects any instruction carrying more
    than one sem wait ("Too many sync wait commands"). Move extra waits onto
    engine NOPs inserted immediately before the instruction."""
    import bass_rust
    for f in nc.m.functions:
        for blk in f.blocks:
            il = blk.instructions
            i = 0
            while i < len(il):
                inst = il[i]
                si = inst.sync_info
                if si is not None and si.on_wait is not None and len(si.on_wait) > 1:
                    waits = list(si.on_wait)
                    si.on_wait = waits[-1:]
                    for w in waits[:-1]:
                        nop = nc.engines[inst.engine].nop(nofuse=True).ins
                        cur = nc.cur_bb.bb.instructions
                        assert cur[-1] is nop
                        cur.pop()
                        nop.sync_info = bass_rust.SyncInfo(on_wait=[w], on_update=[])
                        il.insert(i, nop)
                        i += 1
                i += 1


# ---------------- host preprocessing ----------------
def prep(inputs, n=N, ncores=NCORES):
    """Returns (in_maps, B). Shapes are data-driven only through B."""
    nloc = n // ncores
    st_n = (nloc + P - 1) // P
    x = np.ascontiguousarray(np.asarray(inputs["x"], dtype=np.float32))
    ei = np.asarray(inputs["edge_index"])
    W1_l = np.asarray(inputs["W1_l"], np.float32)
    W1_r = np.asarray(inputs["W1_r"], np.float32)
    b1 = np.asarray(inputs["b1"], np.float32)
    att1 = np.asarray(inputs["att1"], np.float32)
    W2_l = np.asarray(inputs["W2_l"], np.float32)
    W2_r = np.asarray(inputs["W2_r"], np.float32)
    b2 = np.asarray(inputs["b2"], np.float32)
    att2 = np.asarray(inputs["att2"], np.float32)

    # self-loops are NOT packed as edges: their sources are supertile-local,
    # so the device adds one identity-scatter block per supertile instead
    s_all = ei[0].astype(np.int64)
    d_all = ei[1].astype(np.int64)
    order = np.argsort(d_all, kind="stable")
    s_all = s_all[order].astype(np.int32)
    d_all = d_all[order].astype(np.int32)

    bounds = np.searchsorted(d_all, np.arange(ncores + 1) * nloc)
    # first pass: per-supertile block counts (max over cores)
    blocks = np.ones(st_n, np.int64)
    core_data = []
    for c in range(ncores):
        lo, hi = bounds[c], bounds[c + 1]
        dl = d_all[lo:hi] - c * nloc
        sl = s_all[lo:hi]
        stc = dl >> 7
        counts = np.bincount(stc, minlength=st_n)
        blocks = np.maximum(blocks, (counts + P - 1) // P)
        core_data.append((dl, sl, stc, counts))
    B = int(blocks.max())

    # weights / consts
    A1 = np.zeros((H1, HEADS), np.float32)
    for h in range(HEADS):
        A1[h * HID:(h + 1) * HID, h] = att1[h]
    Wa1_l = 0.2 * (W1_l @ A1)
    Wa1_r = 0.2 * (W1_r @ A1)
    W1cat = np.concatenate([W1_l, Wa1_l, W1_r, Wa1_r], axis=1).astype(np.float32)
    A2 = att2.reshape(OUT, 1).astype(np.float32)
    Wa2_l = 0.2 * (W2_l @ A2)
    Wa2_r = 0.2 * (W2_r @ A2)
    zc = np.zeros((H1, 1), np.float32)
    W2cat = np.concatenate([W2_l, Wa2_l, zc, W2_r, Wa2_r, zc], axis=1).astype(np.float32)
    att1r = np.tile(0.8 * att1.reshape(1, H1), (P, 1)).astype(np.float32)
    att2r = np.tile(0.8 * att2.reshape(1, OUT), (P, 1)).astype(np.float32)
    b1r = np.tile(b1.reshape(1, H1), (P, 1)).astype(np.float32)
    b2r = np.tile(b2.reshape(1, OUT), (P, 1)).astype(np.float32)
    colix = np.tile(np.arange(P, dtype=np.float32), (P, 1))
    rowix = np.arange(P, dtype=np.float32).reshape(P, 1)
    import ml_dtypes
    bf = ml_dtypes.bfloat16
    W1cat = W1cat.astype(bf)
    xT = np.ascontiguousarray(x.T).astype(bf)

    in_maps = []
    for c in range(ncores):
        dl, sl, stc, counts = core_data[c]
        starts = np.zeros(st_n, np.int64)
        starts[1:] = np.cumsum(counts)[:-1]
        pos = np.arange(len(dl)) - starts[stc]
        bb = (pos >> 7).astype(np.int64)
        ee = (pos & 127).astype(np.int64)
        esrc = np.zeros((st_n, P, B), np.int32)
        ek = np.full((st_n, P, B), PAD_SLOT, np.float32)
        esrc[stc, ee, bb] = sl
        ek[stc, ee, bb] = (dl - (stc << 7)).astype(np.float32)
        edat = np.concatenate([esrc, ek.view(np.int32)], axis=2)
        # ek transposed to a flat row per supertile: ekt[st, b*128+p] = ek[st, p, b]
        ekt = np.ascontiguousarray(
            ek.transpose(0, 2, 1).reshape(st_n, B * P))
        in_maps.append({
            "xT": xT,
            "xTo": np.ascontiguousarray(x[c * nloc:(c + 1) * nloc].T).astype(bf),
            "W1": W1cat, "W2": W2cat,
            "att1r": att1r, "att2r": att2r,
            "b1r": b1r, "b2r": b2r, "colix": colix,
            "edat": edat, "ekt": ekt, "rowix": rowix,
        })
    return in_maps, blocks


# ---------------- device program ----------------
def build_program(blocks, n=N, ncores=NCORES, grp=3, reps=1):
    blocks = [int(b) for b in blocks]
    B = max(blocks)
    nloc = n // ncores
    st_n = (nloc + P - 1) // P
    last = nloc - (st_n - 1) * P
    w1row = H1 + HEADS          # 132: [xl | a_l]
    w2row = OUT + 2             # 66:  [xl2 | a2l | pad]
    nt_full = (n + P - 1) // P  # dense tiles over all nodes

    nc = bass.Bass()
    xT = nc.dram_tensor("xT", [P, n], BF16, kind="ExternalInput")
    xTo = nc.dram_tensor("xTo", [P, nloc], BF16, kind="ExternalInput")
    W1 = nc.dram_tensor("W1", [P, 2 * w1row], BF16, kind="ExternalInput")
    W2 = nc.dram_tensor("W2", [P, 2 * w2row], F32, kind="ExternalInput")
    att1r = nc.dram_tensor("att1r", [P, H1], F32, kind="ExternalInput")
    att2r = nc.dram_tensor("att2r", [P, OUT], F32, kind="ExternalInput")
    b1r = nc.dram_tensor("b1r", [P, H1], F32, kind="ExternalInput")
    b2r = nc.dram_tensor("b2r", [P, OUT], F32, kind="ExternalInput")
    colix = nc.dram_tensor("colix", [P, P], F32, kind="ExternalInput")
    edat = nc.dram_tensor("edat", [st_n, P, 2 * B], I32, kind="ExternalInput")
    ekt = nc.dram_tensor("ekt", [st_n, B * P], F32, kind="ExternalInput")
    rowix = nc.dram_tensor("rowix", [P, 1], F32, kind="ExternalInput")
    out_loc = nc.dram_tensor("out_loc", [nloc, OUT], F32, kind="ExternalOutput")

    xl1 = nc.dram_tensor("xl1", [n, w1row], BF16)
    xr1 = nc.dram_tensor("xr1", [nloc, w1row], F32)
    hT = nc.dram_tensor("hT", [P, nloc], F32)
    xl2g = nc.dram_tensor("xl2g", [nloc, w2row], BF16)
    xl2f = nc.dram_tensor("xl2f", [nloc, w2row], F32)
    xl2 = nc.dram_tensor("xl2", [n, w2row], BF16, addr_space="Shared")
    xr2 = nc.dram_tensor("xr2", [nloc, w2row], F32)

    cc_sem = nc.alloc_semaphore("cc_sem")

    rep_emit = []

    def edge_layer(tc, pools, consts, table_l, table_r, row_w, dat_w, heads,
                   att_sb, bias_sb, layer, xls_get):
        """Shared edge-phase emitter for both layers."""
        pool, psum_agg, psum_tp, psum_ek, psum_z = pools
        colix_sb, ident = consts["colix"], consts["ident"]
        rowix_sb, ones1 = consts["rowix"], consts["ones1"]
        identb = consts["identb"]
        mrow = dat_w + heads  # matmul rhs width per block
        for st in range(st_n):
            cnt = P if st < st_n - 1 else last
            bst = blocks[st]
            # edge metadata + xr rows load on the idle SP queue so they
            # prefetch during the dense phase instead of queueing behind it
            edt = pool.tile([P, 2 * bst], I32, tag="edt")
            nc.sync.dma_start(
                out=edt[:].rearrange("p (k b) -> p k b", k=2),
                in_=edat[st].rearrange("p (k b) -> p k b", k=2)[:, :, 0:bst])
            es = edt[:, 0:bst]
            ekb = edt[:, bst:2 * bst].bitcast(F32)
            # per-supertile xr rows (slot-indexed); zero tail rows so the
            # S@xr matmul reads clean partitions
            xr_st = pool.tile([P, row_w], F32, tag="xrst")
            if cnt < P:
                nc.gpsimd.memset(xr_st[:], 0.0)
            nc.sync.dma_start(out=xr_st[:cnt, :],
                              in_=table_r[st * P:st * P + cnt, :])
            ektr = pool.tile([1, bst * P], F32, tag="ektr")
            nc.sync.dma_start(out=ektr[:], in_=ekt[st:st + 1, 0:bst * P])
            ps = psum_agg.tile([P, mrow], F32, tag="agg")
            # ---- self-loop block: local contiguous sources, identity
            # scatter — no indirect DMA, no one-hot build ----
            xls = xls_get(st)
            zs = pool.tile([P, row_w], F32, tag="zs")
            nc.vector.tensor_tensor(out=zs[:], in0=xls, in1=xr_st[:],
                                    op=ALU.add)
            r4s = pool.tile([P, row_w], F32, tag="r4s")
            nc.scalar.activation(r4s[:], zs[:], AF.Relu)
            prs = pool.tile([P, dat_w], F32, tag="prs")
            nc.vector.tensor_tensor(out=prs[:], in0=r4s[:, 0:dat_w],
                                    in1=att_sb[:], op=ALU.mult)
            lgs = pool.tile([P, heads], F32, tag="lgs")
            nc.vector.reduce_sum(
                out=lgs[:],
                in_=prs[:].rearrange("p (h c) -> p h c", h=heads),
                axis=mybir.AxisListType.X)
            lgfs = pool.tile([P, heads], F32, tag="lgfs")
            nc.vector.tensor_tensor(out=lgfs[:], in0=lgs[:],
                                    in1=zs[:, dat_w:dat_w + heads], op=ALU.add)
            mgs = pool.tile([P, mrow], F32, tag="mgs")
            nc.scalar.activation(mgs[:, dat_w:dat_w + heads], lgfs[:], AF.Exp)
            cphs = dat_w // heads
            nc.vector.tensor_tensor(
                out=mgs[:, 0:dat_w].rearrange("p (h c) -> p h c", c=cphs),
                in0=zs[:, 0:dat_w].rearrange("p (h c) -> p h c", c=cphs),
                in1=mgs[:, dat_w:dat_w + heads][:, :, None]
                    .to_broadcast([P, heads, cphs]),
                op=ALU.mult)
            nc.tensor.matmul(out=ps[:], lhsT=ident[:], rhs=mgs[:],
                             start=True, stop=False)
            for g0 in range(0, bst, grp):
                gw = min(grp, bst - g0)
                xg = pool.tile([P, gw * row_w], BF16, tag="xg")
                for b in range(gw):
                    nc.gpsimd.indirect_dma_start(
                        out=xg[:, b * row_w:(b + 1) * row_w],
                        out_offset=None, in_=table_l[:],
                        in_offset=bass.IndirectOffsetOnAxis(
                            ap=es[:, g0 + b:g0 + b + 1], axis=0))
                # broadcast slot-id row across partitions, build S^T one-hot
                ekp = psum_ek.tile([P, gw * P], F32, tag="ekp")
                nc.tensor.matmul(out=ekp[:], lhsT=ones1[:],
                                 rhs=ektr[0:1, g0 * P:(g0 + gw) * P],
                                 start=True, stop=True)
                s4t = pool.tile([P, gw * P], F32, tag="s4t")
                nc.vector.tensor_tensor(
                    out=s4t[:].rearrange("p (g q) -> p g q", q=P),
                    in0=rowix_sb[:, :, None].to_broadcast([P, gw, P]),
                    in1=ekp[:].rearrange("p (g q) -> p g q", q=P),
                    op=ALU.is_equal)
                # z = S@xr + I@xg accumulated in PSUM (no DVE add needed)
                zp = psum_z.tile([P, gw * row_w], F32, tag="zp")
                for b in range(gw):
                    sl = zp[:, b * row_w:(b + 1) * row_w]
                    nc.tensor.matmul(out=sl,
                                     lhsT=s4t[:, b * P:(b + 1) * P],
                                     rhs=xr_st[:], start=True, stop=False)
                    nc.tensor.matmul(out=sl, lhsT=identb[:],
                                     rhs=xg[:, b * row_w:(b + 1) * row_w],
                                     start=False, stop=True)
                r4 = pool.tile([P, gw * row_w], F32, tag="r4")
                nc.scalar.activation(r4[:], zp[:], AF.Relu)
                xz3 = zp[:].rearrange("p (g w) -> p g w", w=row_w)
                r43 = r4[:].rearrange("p (g w) -> p g w", w=row_w)
                pr = pool.tile([P, gw * dat_w], F32, tag="pr")
                nc.vector.tensor_tensor(
                    out=pr[:].rearrange("p (g w) -> p g w", w=dat_w),
                    in0=r43[:, :, 0:dat_w],
                    in1=att_sb[:, None, :].to_broadcast([P, gw, dat_w]),
                    op=ALU.mult)
                lg = pool.tile([P, gw * heads], F32, tag="lg")
                nc.vector.reduce_sum(
                    out=lg[:].rearrange("p (g h) -> p g h", h=heads),
                    in_=pr[:].rearrange("p (g h c) -> p g h c",
                                        h=heads, c=dat_w // heads),
                    axis=mybir.AxisListType.X)
                lgf = pool.tile([P, gw * heads], F32, tag="lgf")
                nc.vector.tensor_tensor(
                    out=lgf[:].rearrange("p (g h) -> p g h", h=heads),
                    in0=lg[:].rearrange("p (g h) -> p g h", h=heads),
                    in1=xz3[:, :, dat_w:dat_w + heads],
                    op=ALU.add)
                mg = pool.tile([P, gw * mrow], F32, tag="mg")
                mg3 = mg[:].rearrange("p (g w) -> p g w", w=mrow)
                nc.scalar.activation(
                    mg3[:, :, dat_w:dat_w + heads],
                    lgf[:].rearrange("p (g h) -> p g h", h=heads),
                    AF.Exp)
                cph = dat_w // heads
                nc.vector.tensor_tensor(
                    out=mg[:].rearrange("p (g m) -> p g m", m=mrow)
                        [:, :, 0:dat_w].rearrange("p g (h c) -> p g h c", c=cph),
                    in0=xz3[:, :, 0:dat_w].rearrange("p g (h c) -> p g h c", c=cph),
                    in1=mg3[:, :, dat_w:dat_w + heads][:, :, :, None]
                        .to_broadcast([P, gw, heads, cph]),
                    op=ALU.mult)
                s4 = pool.tile([P, gw * P], F32, tag="s4")
                nc.vector.tensor_tensor(
                    out=s4[:].rearrange("p (g q) -> p g q", q=P),
                    in0=colix_sb[:, None, :].to_broadcast([P, gw, P]),
                    in1=ekb[:, g0:g0 + gw, None].to_broadcast([P, gw, P]),
                    op=ALU.is_equal)
                for b in range(gw):
                    nc.tensor.matmul(
                        out=ps[:],
                        lhsT=s4[:, b * P:(b + 1) * P],
                        rhs=mg[:, b * mrow:(b + 1) * mrow],
                        start=False, stop=(g0 + b == bst - 1))
            # ---- epilogue ----
            # z = xl[src] + xr[dst]; per node i the aggregate is
            # sum(w*xl_src) + xr_i*sum(w), so subtract xr_i*sum(w).
            xrn = xr_st[:, 0:dat_w]
            dn = pool.tile([P, heads], F32, tag="dn")
            nc.vector.tensor_scalar_add(dn[:], ps[:, dat_w:dat_w + heads], 1e-16)
            r0 = pool.tile([P, heads], F32, tag="r0")
            nc.vector.reciprocal(r0[:], dn[:])
            e1 = pool.tile([P, heads], F32, tag="e1")
            nc.vector.tensor_tensor(out=e1[:], in0=r0[:], in1=dn[:], op=ALU.mult)
            t2 = pool.tile([P, heads], F32, tag="t2")
            nc.vector.tensor_scalar(out=t2[:], in0=e1[:], scalar1=-1.0,
                                    scalar2=2.0, op0=ALU.mult, op1=ALU.add)
            r1 = pool.tile([P, heads], F32, tag="r1")
            nc.vector.tensor_tensor(out=r1[:], in0=r0[:], in1=t2[:], op=ALU.mult)
            cor = pool.tile([P, dat_w], F32, tag="cor")
            nc.vector.tensor_tensor(
                out=cor[:].rearrange("p (h c) -> p h c", c=cph),
                in0=xrn[:].rearrange("p (h c) -> p h c", c=cph),
                in1=dn[:, :, None].to_broadcast([P, heads, cph]),
                op=ALU.mult)
            sub = pool.tile([P, dat_w], F32, tag="sub")
            nc.vector.tensor_tensor(out=sub[:], in0=ps[:, 0:dat_w], in1=cor[:],
                                    op=ALU.subtract)
            ob = pool.tile([P, dat_w], F32, tag="ob")
            nc.vector.tensor_tensor(
                out=ob[:].rearrange("p (h c) -> p h c", c=cph),
                in0=sub[:].rearrange("p (h c) -> p h c", c=cph),
                in1=r1[:, :, None].to_broadcast([P, heads, cph]),
                op=ALU.mult)
            ob2 = pool.tile([P, dat_w], F32, tag="ob2")
            nc.vector.tensor_tensor(out=ob2[:], in0=ob[:], in1=bias_sb[:], op=ALU.add)
            if layer == 1:
                mn = pool.tile([P, dat_w], F32, tag="mn")
                nc.vector.tensor_scalar_min(mn[:], ob2[:], 0.0)
                ex = pool.tile([P, dat_w], F32, tag="ex")
                nc.scalar.activation(ex[:], mn[:], AF.Exp)
                rl = pool.tile([P, dat_w], F32, tag="rl")
                nc.scalar.activation(rl[:], ob2[:], AF.Relu)
                sm = pool.tile([P, dat_w], F32, tag="sm")
                nc.vector.tensor_tensor(out=sm[:], in0=ex[:], in1=rl[:], op=ALU.add)
                he = pool.tile([P, dat_w], F32, tag="he")
                nc.vector.tensor_scalar_add(he[:], sm[:], -1.0)
                tp = psum_tp.tile([P, P], F32, tag="tp")
                nc.tensor.transpose(out=tp[:], in_=he[:], identity=ident[:])
                ts = pool.tile([P, P], F32, tag="ts")
                nc.scalar.copy(out=ts[:], in_=tp[:])
                nc.sync.dma_start(out=hT[:, st * P:st * P + cnt], in_=ts[:, :cnt])
            else:
                nc.sync.dma_start(out=out_loc[st * P:st * P + cnt, :],
                                  in_=ob2[:cnt, :])

    for rep in range(reps):
        # one TileContext: dense1 + edges1 + dense2 + AllGather + edges2.
        # Tile's shadow-memory tracks DRAM deps, so the collective and both
        # edge phases order correctly while unrelated work overlaps.
        with TileContext(nc) as tc:
            with tc.tile_pool(name="const", bufs=1) as cpool, \
                 tc.tile_pool(name="work", bufs=4) as pool, \
                 tc.tile_pool(name="dense", bufs=4) as dpool, \
                 tc.tile_pool(name="selfrows", bufs=1) as spool, \
                 tc.tile_pool(name="pagg", bufs=2, space="PSUM") as psum_agg, \
                 tc.tile_pool(name="ptp", bufs=1, space="PSUM") as psum_tp, \
                 tc.tile_pool(name="pek", bufs=1, space="PSUM") as psum_ek, \
                 tc.tile_pool(name="pz", bufs=2, space="PSUM") as psum_z, \
                 tc.tile_pool(name="pd", bufs=2, space="PSUM") as psum_d:
                w1_sb = cpool.tile([P, 2 * w1row], BF16)
                nc.sync.dma_start(out=w1_sb[:], in_=W1[:])
                w2_sb = cpool.tile([P, 2 * w2row], F32)
                nc.sync.dma_start(out=w2_sb[:], in_=W2[:])
                att1_sb = cpool.tile([P, H1], F32)
                nc.sync.dma_start(out=att1_sb[:], in_=att1r[:])
                att2_sb = cpool.tile([P, OUT], F32)
                nc.sync.dma_start(out=att2_sb[:], in_=att2r[:])
                b1_sb = cpool.tile([P, H1], F32)
                nc.sync.dma_start(out=b1_sb[:], in_=b1r[:])
                b2_sb = cpool.tile([P, OUT], F32)
                nc.sync.dma_start(out=b2_sb[:], in_=b2r[:])
                colix_sb = cpool.tile([P, P], F32)
                nc.sync.dma_start(out=colix_sb[:], in_=colix[:])
                ident = cpool.tile([P, P], F32)
                make_identity(nc, ident[:])
                rowix_sb = cpool.tile([P, 1], F32)
                nc.sync.dma_start(out=rowix_sb[:], in_=rowix[:])
                ones1 = cpool.tile([1, P], F32)
                nc.gpsimd.memset(ones1[:], 1.0)
                identb = cpool.tile([P, P], BF16)
                nc.scalar.copy(out=identb[:], in_=ident[:])
                consts = {"colix": colix_sb, "ident": ident,
                          "rowix": rowix_sb, "ones1": ones1,
                          "identb": identb}

                # dense-1: xl1 (all nodes), batched 4 tiles per DMA
                nb = 8
                for t0 in range(0, nt_full, nb):
                    k_n = min(nb, nt_full - t0)
                    cols_all = min(P * k_n, n - t0 * P)
                    xt = dpool.tile([P, P * k_n], BF16, tag="xt4")
                    nc.scalar.dma_start(out=xt[:, :cols_all],
                                        in_=xT[:, t0 * P:t0 * P + cols_all])
                    sb = dpool.tile([P, k_n * w1row], BF16, tag="sbd4")
                    for k in range(k_n):
                        cols = min(P, n - (t0 + k) * P)
                        psd = psum_d.tile([cols, w1row], F32, tag="psd")
                        nc.tensor.matmul(out=psd[:],
                                         lhsT=xt[:, k * P:k * P + cols],
                                         rhs=w1_sb[:, 0:w1row],
                                         start=True, stop=True)
                        # alternate PSUM drains between Act and DVE so the
                        # copy stream isn't the dense-phase serial driver
                        dst = sb[:cols, k * w1row:(k + 1) * w1row]
                        if k % 2 == 0:
                            nc.scalar.copy(out=dst, in_=psd[:])
                        else:
                            nc.vector.tensor_scalar_add(dst, psd[:], 0.0)
                    rows = min(P * k_n, n - t0 * P)
                    if rows == P * k_n:
                        nc.sync.dma_start(
                            out=xl1[t0 * P:t0 * P + rows, :]
                                .rearrange("(k p) w -> p k w", p=P),
                            in_=sb[:].rearrange("p (k w) -> p k w", w=w1row))
                    else:
                        # ragged tail: per-block writes
                        for k in range(k_n):
                            cols = min(P, n - (t0 + k) * P)
                            nc.sync.dma_start(
                                out=xl1[(t0 + k) * P:(t0 + k) * P + cols, :],
                                in_=sb[:cols, k * w1row:(k + 1) * w1row])
                # local pass: xr rows (fp32, to DRAM) + self-loop xl rows
                # (bf16, kept in SBUF for the per-supertile identity block)
                xls1 = []
                for t in range(st_n):
                    cols = P if t < st_n - 1 else last
                    xt = dpool.tile([P, cols], BF16, tag="xt")
                    nc.scalar.dma_start(out=xt[:], in_=xTo[:, t * P:t * P + cols])
                    psd = psum_d.tile([cols, 2 * w1row], F32, tag="psd")
                    nc.tensor.matmul(out=psd[:], lhsT=xt[:],
                                     rhs=w1_sb[:], start=True, stop=True)
                    sb = dpool.tile([cols, w1row], F32, tag="sbd")
                    nc.scalar.copy(out=sb[:], in_=psd[:, w1row:2 * w1row])
                    nc.sync.dma_start(out=xr1[t * P:t * P + cols, :], in_=sb[:])
                    xsl = spool.tile([P, w1row], F32, tag=f"xsl{t}")
                    if cols < P:
                        nc.gpsimd.memset(xsl[:], 0.0)
                    nc.vector.tensor_scalar_add(xsl[:cols, :],
                                                psd[:, 0:w1row], 0.0)
                    xls1.append(xsl)

                # edges layer 1
                edge_layer(tc, (pool, psum_agg, psum_tp, psum_ek, psum_z),
                           consts, xl1, xr1,
                           w1row, H1, HEADS, att1_sb, b1_sb, layer=1,
                           xls_get=lambda st: xls1[st][:])

                # dense-2: xl2g + xr2 from hT
                for t in range(st_n):
                    cols = P if t < st_n - 1 else last
                    xh = dpool.tile([P, cols], F32, tag="xt")
                    nc.scalar.dma_start(out=xh[:], in_=hT[:, t * P:t * P + cols])
                    psd2 = psum_d.tile([cols, 2 * w2row], F32, tag="psd")
                    nc.tensor.matmul(out=psd2[:], lhsT=xh[:], rhs=w2_sb[:],
                                     start=True, stop=True)
                    sb2l = dpool.tile([cols, w2row], BF16, tag="sbdl")
                    nc.scalar.copy(out=sb2l[:], in_=psd2[:, 0:w2row])
                    sb2f = dpool.tile([cols, w2row], F32, tag="sbdf")
                    nc.scalar.copy(out=sb2f[:], in_=psd2[:, 0:w2row])
                    sb2r = dpool.tile([cols, w2row], F32, tag="sbdr")
                    nc.vector.tensor_scalar_add(sb2r[:],
                                                psd2[:, w2row:2 * w2row], 0.0)
                    nc.sync.dma_start(out=xl2g[t * P:t * P + cols, :],
                                      in_=sb2l[:])
                    nc.sync.dma_start(out=xl2f[t * P:t * P + cols, :],
                                      in_=sb2f[:])
                    nc.sync.dma_start(out=xr2[t * P:t * P + cols, :],
                                      in_=sb2r[:])

        # ---- AllGather xl2g -> xl2 (between TileContexts; raw sem) ----
        nc.gpsimd.collective_compute(
            "AllGather", ALU.bypass,
            replica_groups=[list(range(ncores))],
            ins=[xl2g[:]], outs=[xl2[:]],
        ).then_inc(cc_sem)
        nc.gpsimd.wait_ge(cc_sem, rep + 1)

        # ---- TC2: edges layer 2 ----
        with TileContext(nc) as tc:
            with tc.tile_pool(name="const2", bufs=1) as cpool, \
                 tc.tile_pool(name="work2", bufs=4) as pool, \
                 tc.tile_pool(name="pagg2", bufs=2, space="PSUM") as psum_agg, \
                 tc.tile_pool(name="pek2", bufs=2, space="PSUM") as psum_ek, \
                 tc.tile_pool(name="pz2", bufs=3, space="PSUM") as psum_z:
                psum_tp = psum_z  # unused in layer 2
                att2_sb = cpool.tile([P, OUT], F32)
                nc.sync.dma_start(out=att2_sb[:], in_=att2r[:])
                b2_sb = cpool.tile([P, OUT], F32)
                nc.sync.dma_start(out=b2_sb[:], in_=b2r[:])
                colix_sb = cpool.tile([P, P], F32)
                nc.sync.dma_start(out=colix_sb[:], in_=colix[:])
                ident = cpool.tile([P, P], F32)
                make_identity(nc, ident[:])
                rowix_sb = cpool.tile([P, 1], F32)
                nc.sync.dma_start(out=rowix_sb[:], in_=rowix[:])
                ones1 = cpool.tile([1, P], F32)
                nc.gpsimd.memset(ones1[:], 1.0)
                identb = cpool.tile([P, P], BF16)
                nc.scalar.copy(out=identb[:], in_=ident[:])
                consts = {"colix": colix_sb, "ident": ident,
                          "rowix": rowix_sb, "ones1": ones1,
                          "identb": identb}
                def xls_get2(st):
                    cnt = P if st < st_n - 1 else last
                    t = pool.tile([P, w2row], F32, tag="xsl2")
                    if cnt < P:
                        nc.gpsimd.memset(t[:], 0.0)
                    nc.sync.dma_start(out=t[:cnt, :],
                                      in_=xl2f[st * P:st * P + cnt, :])
                    return t[:]

                edge_layer(tc, (pool, psum_agg, psum_tp, psum_ek, psum_z),
                           consts, xl2, xr2,
                           w2row, OUT, 1, att2_sb, b2_sb, layer=2,
                           xls_get=xls_get2)

    return nc


# ---------------- entry point ----------------
def kernel(**inputs) -> np.ndarray:
    in_maps, blocks = prep(inputs)
    nc = build_program(blocks)
    split_multi_waits(nc)
    res = run_bass_kernel_spmd(nc, in_maps, list(range(NCORES)))
    out = np.concatenate([res.results[c]["out_loc"] for c in range(NCORES)], axis=0)
    return out.astype(np.float32)

